# revision 24
# baseline (speedup 1.0000x reference)
"""DharmaAttention TRN2 kernel (fused single-pass, bf16, v3).

Full-input contract: kernel(**inputs) takes the unsharded inputs and returns
the full [2, 2048, 2048] output.

Sharding (8 cores): 2-way data-parallel over batch x 4-way tensor-parallel
over head groups (4 heads of head_dim 128 per core). Wq/Wk/Wv are split
column-wise (output channels) per head group, Wo row-wise; each core produces
a partial output projection for its batch element and the host sums the 4
partials per batch.

v3 changes vs v2:
  - reciprocal_approx_fast instead of reciprocal (3.4us -> 0.7us DVE op that
    was head-of-line blocking the diag mask multiplies -> PE stalls).
  - Diagonal blocks compute only the valid (causal) column range; the
    triangular mask shrinks to a single [128,128] constant applied to one
    sub-block per diagonal block.
  - One PSUM pool with shared tags across phases (no pool-transition
    barriers at phase boundaries).
  - Startup DMAs split/ordered so the first V matmul starts early.
  - bf16 output staged per [128,512] block (halves output DMA, short tail).

Per-core layouts (host-side prep):
  xT   [2048, 2048] bf16  hidden_states[b].T       (contraction dim on partitions)
  wqT  [2048, 512]  bf16  Wq[rows of group].T      (same for wkT, wvT)
  woc  [512, 2048]  bf16  Wo[:, cols of group].T
  cosT [128, 2048]  f32   rope cos table, [d, s]
  sinN [128, 2048]  f32   rows 0:64 = -sin, rows 64:128 = +sin, [d, s]
  tri  [128, 128]   bf16  tri[p, s] = 1 if s >= p (in-block causal mask)
Output:
  yT   [2048, 2048] bf16  partial (Wo row-shard) output, transposed [o, s]

Softmax skips the max subtraction: scores are O(+-6), exp is safe in fp32,
and softmax is shift-invariant so the result matches the reference.
"""

import math
import sys

sys.path.insert(0, "/opt/trn_rl_repo")

import numpy as np

B = 2
S = 2048
H = 2048
NH = 16
HD = 128
THETA = 10000.0
G = 4  # heads per core (tensor-parallel group size NH / 4)
GC = G * HD  # channels per core = 512
NHT = H // 128  # 16 contraction tiles
SC = 512  # projection seq chunk
NSC = S // SC  # 4
QC = 512  # attention q chunk
NQC = S // QC  # 4
NKB = S // 128  # 16 k blocks
INV_SQRT_HD = 1.0 / math.sqrt(HD)

_prog_cache = {}

# test-harness hooks (the grading path leaves these at defaults)
TRACE = False
LAST_RESULTS = None


def _split_multi_waits(nc):
    """The walrus build here accepts at most ONE sync wait per instruction
    ('Too many sync wait commands'). Hoist extra on_wait entries into no-op
    instructions inserted just before, on the same engine."""
    import concourse.mybir as mybir

    for f in nc.m.functions:
        for b in f.blocks:
            out = []
            changed = False
            for inst in b.instructions:
                si = getattr(inst, "sync_info", None)
                waits = list(si.on_wait) if si is not None and si.on_wait else []
                if len(waits) > 1:
                    for k, w in enumerate(waits[:-1]):
                        nop = mybir.InstNoOp(
                            name=f"{inst.name}-w{k}",
                            sync_info=mybir.SyncInfo(on_wait=[w], on_update=[]),
                        )
                        nop.engine = inst.engine
                        out.append(nop)
                    inst.sync_info = mybir.SyncInfo(
                        on_wait=[waits[-1]], on_update=list(si.on_update or [])
                    )
                    changed = True
                out.append(inst)
            if changed:
                b.instructions = out
    return nc


def _build_nc():
    import concourse.bass as bass
    import concourse.mybir as mybir
    import concourse.tile as tile

    F32 = mybir.dt.float32
    BF16 = mybir.dt.bfloat16
    MULT = mybir.AluOpType.mult
    ADD = mybir.AluOpType.add
    DIV = mybir.AluOpType.divide
    EXP = mybir.ActivationFunctionType.Exp

    nc = bass.Bass("TRN2", target_bir_lowering=False, debug=False)

    xT = nc.dram_tensor("xT", [H, S], BF16, kind="ExternalInput").ap()
    wqT = nc.dram_tensor("wqT", [H, GC], BF16, kind="ExternalInput").ap()
    wkT = nc.dram_tensor("wkT", [H, GC], BF16, kind="ExternalInput").ap()
    wvT = nc.dram_tensor("wvT", [H, GC], BF16, kind="ExternalInput").ap()
    woc = nc.dram_tensor("woc", [GC, H], BF16, kind="ExternalInput").ap()
    cosT_d = nc.dram_tensor("cosT", [HD, S], BF16, kind="ExternalInput").ap()
    sinN_d = nc.dram_tensor("sinN", [HD, S], BF16, kind="ExternalInput").ap()
    tri_d = nc.dram_tensor("tri", [128, 128], BF16, kind="ExternalInput").ap()
    yT = nc.dram_tensor("yT", [H, S], BF16, kind="ExternalOutput").ap()

    with tile.TileContext(nc) as tc:
        with (
            tc.tile_pool(name="consts", bufs=1) as consts,
            tc.tile_pool(name="qkv", bufs=1) as qkv,
            tc.tile_pool(name="wpool", bufs=1) as wpool,
            tc.tile_pool(name="xpool", bufs=2) as xpool,
            tc.tile_pool(name="rpool", bufs=3) as rpool,
            tc.tile_pool(name="prpool", bufs=4) as prpool,
            tc.tile_pool(name="bcpool", bufs=2) as bcpool,
            tc.tile_pool(name="ystage", bufs=4) as ystage,
            tc.tile_pool(name="ps", bufs=1, space="PSUM") as ps,
        ):
            # persistent SBUF state for the whole kernel
            cosT = consts.tile([HD, S], BF16)
            sinN = consts.tile([HD, S], BF16)
            tri = consts.tile([128, 128], BF16)
            ones_f = consts.tile([128, 128], F32)
            ones_mat = consts.tile([128, 128], BF16)
            woc_sb = consts.tile([128, G, H], BF16, tag="woc")

            q_all = qkv.tile([128, G, S], BF16, tag="q")  # [d, h, s]
            k_all = qkv.tile([128, G, S], BF16, tag="k")  # [d, h, s]
            v_all = qkv.tile([128, NKB, GC], BF16, tag="v")  # [s_in_blk, blk, (h d)]
            outh = qkv.tile([128, G, S], BF16, tag="o")  # [d, h, s]

            wv_sb = wpool.tile([128, NHT, GC], BF16, tag="wv")
            wq_sb = wpool.tile([128, NHT, GC], BF16, tag="wq")
            wk_sb = wpool.tile([128, NHT, GC], BF16, tag="wk")
            x0 = xpool.tile([128, NHT, SC], BF16, tag="x")

            # startup DMAs, interleaved so the first V matmul group (which
            # needs all of wv + x0) completes as early as possible, with
            # later-needed tensors queued behind.
            wvr = wvT.rearrange("(t p) o -> p t o", p=128)
            x0r = xT[:, 0:SC].rearrange("(t p) s -> p t s", p=128)
            for c in range(4):
                tsl = slice(4 * c, 4 * c + 4)
                nc.sync.dma_start(out=wv_sb[:, tsl, :], in_=wvr[:, tsl, :])
                nc.sync.dma_start(out=x0[:, tsl, :], in_=x0r[:, tsl, :])
            # wq/wk split per head slice, interleaved in the order phase A
            # consumes them (Q head 0, K head 0, Q head 1, ...)
            wqr = wqT.rearrange("(t p) o -> p t o", p=128)
            wkr = wkT.rearrange("(t p) o -> p t o", p=128)
            for h in range(G):
                hsl = slice(h * 128, (h + 1) * 128)
                nc.sync.dma_start(out=wq_sb[:, :, hsl], in_=wqr[:, :, hsl])
                nc.sync.dma_start(out=wk_sb[:, :, hsl], in_=wkr[:, :, hsl])
            nc.sync.dma_start(out=cosT, in_=cosT_d)
            nc.sync.dma_start(out=sinN, in_=sinN_d)
            nc.vector.memset(ones_f, 1.0)
            nc.vector.tensor_copy(ones_mat, ones_f)

            # ---------------- Phase A: QKV projections + RoPE (one x pass) ---
            for sc in range(NSC):
                ssl = slice(sc * SC, (sc + 1) * SC)
                if sc == 0:
                    x_sb = x0
                else:
                    x_sb = xpool.tile([128, NHT, SC], BF16, tag="x")
                    nc.sync.dma_start(
                        out=x_sb, in_=xT[:, ssl].rearrange("(t p) s -> p t s", p=128)
                    )
                # V projection: x block stationary -> [s, (h d)] orientation
                for st2 in range(SC // 128):
                    st = sc * (SC // 128) + st2
                    pv = ps.tile([128, GC], F32, tag="a", bufs=2)
                    for ht in range(NHT):
                        nc.tensor.matmul(
                            pv,
                            x_sb[:, ht, st2 * 128 : (st2 + 1) * 128],
                            wv_sb[:, ht, :],
                            start=(ht == 0),
                            stop=(ht == NHT - 1),
                        )
                    nc.scalar.copy(v_all[:, st, :], pv)
                if sc == 0:
                    # Stagger the late-needed woc/tri DMAs behind chunk 0's V
                    # work (WAW dep via dummy writes that depend on v_all) so
                    # startup DMA bandwidth goes to wv/x0/wq/wk/cos/sin.
                    nc.vector.tensor_copy(woc_sb[0:1, 0, 0:1], v_all[0:1, 3, 0:1])
                    nc.vector.tensor_copy(tri[0:1, 0:1], v_all[0:1, 3, 0:1])
                    nc.sync.dma_start(out=tri, in_=tri_d)
                    nc.sync.dma_start(
                        out=woc_sb, in_=woc.rearrange("(c p) o -> p c o", p=128)
                    )
                # Q/K projections: w block stationary -> [d, s] orientation.
                # Q and K of a head share one [128,1024] PSUM pair tile so
                # phase A uses the same PSUM tag zones as phase B.
                for h in range(G):
                    pqk2 = ps.tile([128, 2 * SC], F32, tag="b", bufs=2)
                    for off, w_sb in ((0, wq_sb), (SC, wk_sb)):
                        pqk = pqk2[:, off : off + SC]
                        for ht in range(NHT):
                            nc.tensor.matmul(
                                pqk,
                                w_sb[:, ht, h * 128 : (h + 1) * 128],
                                x_sb[:, ht, :],
                                start=(ht == 0),
                                stop=(ht == NHT - 1),
                            )
                    for off, dst in ((0, q_all), (SC, k_all)):
                        pqk = pqk2[:, off : off + SC]
                        # RoPE: dst = pqk * cos + rot_half(pqk) * sin
                        tmp = rpool.tile([128, SC], F32, tag="tmp")
                        nc.vector.tensor_tensor(
                            out=tmp[0:64, :], in0=pqk[64:128, :],
                            in1=sinN[0:64, ssl], op=MULT,
                        )
                        nc.vector.tensor_tensor(
                            out=tmp[64:128, :], in0=pqk[0:64, :],
                            in1=sinN[64:128, ssl], op=MULT,
                        )
                        cpart = rpool.tile([128, SC], F32, tag="cpart")
                        nc.vector.tensor_tensor(
                            out=cpart, in0=pqk, in1=cosT[:, ssl], op=MULT
                        )
                        nc.vector.tensor_tensor(
                            out=dst[:, h, ssl], in0=cpart, in1=tmp, op=ADD
                        )

            # ---------------- Phase B: attention (all SBUF-resident) ---------
            # k-blocks are processed in PAIRS packed into one [128, 1024]
            # PSUM tile: one exp instruction per pair (halves the Act-engine
            # per-instruction overhead, Act is the phase-B co-bottleneck).
            # Diagonal pairs pack only the causally-valid column ranges.
            # The pair stream is software-pipelined D=2 deep across chunk and
            # head boundaries so the PE never waits on exp latency.
            pairs = []  # (h, qc, kp, npair)
            for h in range(G):
                for qc in range(NQC):
                    npair = 2 * qc + 2
                    for kp in range(npair):
                        pairs.append((h, qc, kp, npair))
            D = 2
            meta = {}  # pair idx -> (pr, halves)
            acc = {}  # (h, qc) -> (po, pbs)

            def front(j):
                h, qc, kp, npair = pairs[j]
                psc = ps.tile([128, 2 * QC], F32, tag="b", bufs=2)
                pr = prpool.tile([128, 2 * QC], BF16, tag="pr")
                halves = []
                off = 0
                for half in range(2):
                    ki = 2 * kp + half
                    m = ki - 4 * qc  # >= 0 on diagonal blocks
                    c0 = max(m, 0) * 128  # valid col start within chunk
                    w = QC - c0
                    nc.tensor.matmul(
                        psc[:, off : off + w],
                        k_all[:, h, ki * 128 : (ki + 1) * 128],
                        q_all[:, h, qc * QC + c0 : (qc + 1) * QC],
                        start=True,
                        stop=True,
                    )
                    halves.append((ki, m, c0, w, off))
                    off += w
                nc.scalar.activation(
                    pr[:, 0:off], psc[:, 0:off], EXP, scale=INV_SQRT_HD
                )
                for ki, m, c0, w, o in halves:
                    if m >= 0:
                        # triangular mask on the partial sub-block, on the
                        # (otherwise idle) Pool engine
                        nc.gpsimd.tensor_tensor(
                            out=pr[:, o : o + 128],
                            in0=pr[:, o : o + 128],
                            in1=tri, op=MULT,
                        )
                meta[j] = (pr, halves)

            def back(j):
                h, qc, kp, npair = pairs[j]
                hd = slice(h * 128, (h + 1) * 128)
                nk = 4 * qc + 4
                if kp == 0:
                    acc[(h, qc)] = (
                        ps.tile([128, QC], F32, tag="a", bufs=2, name="po"),
                        ps.tile([128, QC], F32, tag="c", bufs=2, name="pbs"),
                    )
                po, pbs = acc[(h, qc)]
                pr, halves = meta.pop(j)
                for ki, m, c0, w, o in halves:
                    nc.tensor.matmul(
                        po[:, c0:QC], v_all[:, ki, hd], pr[:, o : o + w],
                        start=(ki == 0), stop=(ki == nk - 1),
                    )
                    nc.tensor.matmul(
                        pbs[:, c0:QC], ones_mat, pr[:, o : o + w],
                        start=(ki == 0), stop=(ki == nk - 1),
                    )
                if kp == npair - 1:
                    # Act stages po out of PSUM right away (frees the bank);
                    # normalize on DVE behind its own reciprocal — nothing
                    # upstream depends on outh until phase C.
                    pof = bcpool.tile([128, QC], F32, tag="pof")
                    nc.scalar.copy(pof, po)
                    bc = bcpool.tile([128, QC], F32, tag="bc")
                    nc.vector.reciprocal(out=bc, in_=pbs)
                    nc.vector.tensor_tensor(
                        out=outh[:, h, slice(qc * QC, (qc + 1) * QC)],
                        in0=pof, in1=bc, op=MULT,
                    )

            for j in range(len(pairs) + D):
                if j < len(pairs):
                    front(j)
                if j >= D:
                    back(j - D)

            # ---------------- Phase C: output projection ---------------------
            # sch outer so the first 16 py groups only need outh columns that
            # finished early in phase B
            for sch in range(NQC):
                ssl = slice(sch * QC, (sch + 1) * QC)
                for ot in range(NHT):
                    py = ps.tile([128, QC], F32, tag="a", bufs=2)
                    for h in range(G):
                        nc.tensor.matmul(
                            py,
                            woc_sb[:, h, ot * 128 : (ot + 1) * 128],
                            outh[:, h, ssl],
                            start=(h == 0),
                            stop=(h == G - 1),
                        )
                    ysf = ystage.tile([128, QC], BF16)
                    nc.scalar.copy(ysf, py)
                    nc.scalar.dma_start(
                        out=yT[ot * 128 : (ot + 1) * 128, ssl], in_=ysf
                    )
    _split_multi_waits(nc)
    return nc


def _host_tables():
    import ml_dtypes

    inv_freq = 1.0 / (THETA ** (np.arange(0, HD, 2, dtype=np.float32) / HD))
    t = np.arange(S, dtype=np.float32)
    freqs = np.einsum("i,j->ij", t, inv_freq)  # [S, 64]
    cos_h = np.cos(freqs).astype(np.float32)  # [S, 64]
    sin_h = np.sin(freqs).astype(np.float32)
    cosT = np.empty((HD, S), np.float32)
    cosT[0:64] = cos_h.T
    cosT[64:128] = cos_h.T
    sinN = np.empty((HD, S), np.float32)
    sinN[0:64] = -sin_h.T
    sinN[64:128] = sin_h.T
    p = np.arange(128)[:, None]
    s = np.arange(128)[None, :]
    tri = (s >= p).astype(ml_dtypes.bfloat16)
    return (
        cosT.astype(ml_dtypes.bfloat16),
        sinN.astype(ml_dtypes.bfloat16),
        tri,
    )


def kernel(hidden_states, Wq, Wk, Wv, Wo):
    import ml_dtypes

    from concourse import bass_utils

    BF = ml_dtypes.bfloat16
    hidden_states = np.asarray(hidden_states, dtype=np.float32)
    Wq = np.asarray(Wq, dtype=np.float32)
    Wk = np.asarray(Wk, dtype=np.float32)
    Wv = np.asarray(Wv, dtype=np.float32)
    Wo = np.asarray(Wo, dtype=np.float32)

    if "nc" not in _prog_cache:
        _prog_cache["nc"] = _build_nc()
    nc = _prog_cache["nc"]

    cosT, sinN, tri = _host_tables()
    in_maps = []
    for c in range(8):
        b, g = divmod(c, 4)
        rows = slice(g * GC, (g + 1) * GC)
        in_maps.append(
            {
                "xT": np.ascontiguousarray(hidden_states[b].T).astype(BF),
                "wqT": np.ascontiguousarray(Wq[rows, :].T).astype(BF),
                "wkT": np.ascontiguousarray(Wk[rows, :].T).astype(BF),
                "wvT": np.ascontiguousarray(Wv[rows, :].T).astype(BF),
                "woc": np.ascontiguousarray(Wo[:, rows].T).astype(BF),
                "cosT": cosT,
                "sinN": sinN,
                "tri": tri,
            }
        )

    res = bass_utils.run_bass_kernel_spmd(
        nc, in_maps, core_ids=list(range(8)), trace=TRACE
    )
    global LAST_RESULTS
    LAST_RESULTS = res

    out = np.zeros((B, S, H), np.float32)
    for c in range(8):
        b = c // 4
        out[b] += res.results[c]["yT"].T.astype(np.float32)
    return out


# revision 25
# speedup vs baseline: 1.0031x; 1.0031x over previous
"""DharmaAttention TRN2 kernel (fused single-pass, bf16, v3).

Full-input contract: kernel(**inputs) takes the unsharded inputs and returns
the full [2, 2048, 2048] output.

Sharding (8 cores): 2-way data-parallel over batch x 4-way tensor-parallel
over head groups (4 heads of head_dim 128 per core). Wq/Wk/Wv are split
column-wise (output channels) per head group, Wo row-wise; each core produces
a partial output projection for its batch element and the host sums the 4
partials per batch.

v3 changes vs v2:
  - reciprocal_approx_fast instead of reciprocal (3.4us -> 0.7us DVE op that
    was head-of-line blocking the diag mask multiplies -> PE stalls).
  - Diagonal blocks compute only the valid (causal) column range; the
    triangular mask shrinks to a single [128,128] constant applied to one
    sub-block per diagonal block.
  - One PSUM pool with shared tags across phases (no pool-transition
    barriers at phase boundaries).
  - Startup DMAs split/ordered so the first V matmul starts early.
  - bf16 output staged per [128,512] block (halves output DMA, short tail).

Per-core layouts (host-side prep):
  xT   [2048, 2048] bf16  hidden_states[b].T       (contraction dim on partitions)
  wqT  [2048, 512]  bf16  Wq[rows of group].T      (same for wkT, wvT)
  woc  [512, 2048]  bf16  Wo[:, cols of group].T
  cosT [128, 2048]  f32   rope cos table, [d, s]
  sinN [128, 2048]  f32   rows 0:64 = -sin, rows 64:128 = +sin, [d, s]
  tri  [128, 128]   bf16  tri[p, s] = 1 if s >= p (in-block causal mask)
Output:
  yT   [2048, 2048] bf16  partial (Wo row-shard) output, transposed [o, s]

Softmax skips the max subtraction: scores are O(+-6), exp is safe in fp32,
and softmax is shift-invariant so the result matches the reference.
"""

import math
import sys

sys.path.insert(0, "/opt/trn_rl_repo")

import numpy as np

B = 2
S = 2048
H = 2048
NH = 16
HD = 128
THETA = 10000.0
G = 4  # heads per core (tensor-parallel group size NH / 4)
GC = G * HD  # channels per core = 512
NHT = H // 128  # 16 contraction tiles
SC = 512  # projection seq chunk
NSC = S // SC  # 4
QC = 512  # attention q chunk
NQC = S // QC  # 4
NKB = S // 128  # 16 k blocks
INV_SQRT_HD = 1.0 / math.sqrt(HD)

_prog_cache = {}

# test-harness hooks (the grading path leaves these at defaults)
TRACE = False
LAST_RESULTS = None


def _split_multi_waits(nc):
    """The walrus build here accepts at most ONE sync wait per instruction
    ('Too many sync wait commands'). Hoist extra on_wait entries into no-op
    instructions inserted just before, on the same engine."""
    import concourse.mybir as mybir

    for f in nc.m.functions:
        for b in f.blocks:
            out = []
            changed = False
            for inst in b.instructions:
                si = getattr(inst, "sync_info", None)
                waits = list(si.on_wait) if si is not None and si.on_wait else []
                if len(waits) > 1:
                    for k, w in enumerate(waits[:-1]):
                        nop = mybir.InstNoOp(
                            name=f"{inst.name}-w{k}",
                            sync_info=mybir.SyncInfo(on_wait=[w], on_update=[]),
                        )
                        nop.engine = inst.engine
                        out.append(nop)
                    inst.sync_info = mybir.SyncInfo(
                        on_wait=[waits[-1]], on_update=list(si.on_update or [])
                    )
                    changed = True
                out.append(inst)
            if changed:
                b.instructions = out
    return nc


def _build_nc():
    import concourse.bass as bass
    import concourse.mybir as mybir
    import concourse.tile as tile

    F32 = mybir.dt.float32
    BF16 = mybir.dt.bfloat16
    MULT = mybir.AluOpType.mult
    ADD = mybir.AluOpType.add
    DIV = mybir.AluOpType.divide
    EXP = mybir.ActivationFunctionType.Exp

    nc = bass.Bass("TRN2", target_bir_lowering=False, debug=False)

    xT = nc.dram_tensor("xT", [H, S], BF16, kind="ExternalInput").ap()
    wqT = nc.dram_tensor("wqT", [H, GC], BF16, kind="ExternalInput").ap()
    wkT = nc.dram_tensor("wkT", [H, GC], BF16, kind="ExternalInput").ap()
    wvT = nc.dram_tensor("wvT", [H, GC], BF16, kind="ExternalInput").ap()
    woc = nc.dram_tensor("woc", [GC, H], BF16, kind="ExternalInput").ap()
    cosT_d = nc.dram_tensor("cosT", [HD, S], BF16, kind="ExternalInput").ap()
    sinN_d = nc.dram_tensor("sinN", [HD, S], BF16, kind="ExternalInput").ap()
    tri_d = nc.dram_tensor("tri", [128, 128], BF16, kind="ExternalInput").ap()
    yT = nc.dram_tensor("yT", [H, S], BF16, kind="ExternalOutput").ap()

    with tile.TileContext(nc) as tc:
        with (
            tc.tile_pool(name="consts", bufs=1) as consts,
            tc.tile_pool(name="qkv", bufs=1) as qkv,
            tc.tile_pool(name="wpool", bufs=1) as wpool,
            tc.tile_pool(name="xpool", bufs=2) as xpool,
            tc.tile_pool(name="rpool", bufs=3) as rpool,
            tc.tile_pool(name="prpool", bufs=4) as prpool,
            tc.tile_pool(name="bcpool", bufs=2) as bcpool,
            tc.tile_pool(name="ystage", bufs=4) as ystage,
            tc.tile_pool(name="ps", bufs=1, space="PSUM") as ps,
        ):
            # persistent SBUF state for the whole kernel
            cosT = consts.tile([HD, S], BF16)
            sinN = consts.tile([HD, S], BF16)
            tri = consts.tile([128, 128], BF16)
            ones_f = consts.tile([128, 128], F32)
            ones_mat = consts.tile([128, 128], BF16)
            woc_sb = consts.tile([128, G, H], BF16, tag="woc")

            q_all = qkv.tile([128, G, S], BF16, tag="q")  # [d, h, s]
            k_all = qkv.tile([128, G, S], BF16, tag="k")  # [d, h, s]
            v_all = qkv.tile([128, NKB, GC], BF16, tag="v")  # [s_in_blk, blk, (h d)]
            outh = qkv.tile([128, G, S], BF16, tag="o")  # [d, h, s]

            wv_sb = wpool.tile([128, NHT, GC], BF16, tag="wv")
            wq_sb = wpool.tile([128, NHT, GC], BF16, tag="wq")
            wk_sb = wpool.tile([128, NHT, GC], BF16, tag="wk")
            x0 = xpool.tile([128, NHT, SC], BF16, tag="x")

            # startup DMAs, interleaved so the first V matmul group (which
            # needs all of wv + x0) completes as early as possible, with
            # later-needed tensors queued behind.
            wvr = wvT.rearrange("(t p) o -> p t o", p=128)
            x0r = xT[:, 0:SC].rearrange("(t p) s -> p t s", p=128)
            for c in range(4):
                tsl = slice(4 * c, 4 * c + 4)
                nc.sync.dma_start(out=wv_sb[:, tsl, :], in_=wvr[:, tsl, :])
                nc.sync.dma_start(out=x0[:, tsl, :], in_=x0r[:, tsl, :])
            # wq/wk split per head slice, interleaved in the order phase A
            # consumes them (Q head 0, K head 0, Q head 1, ...)
            wqr = wqT.rearrange("(t p) o -> p t o", p=128)
            wkr = wkT.rearrange("(t p) o -> p t o", p=128)
            for h in range(G):
                hsl = slice(h * 128, (h + 1) * 128)
                nc.sync.dma_start(out=wq_sb[:, :, hsl], in_=wqr[:, :, hsl])
                nc.sync.dma_start(out=wk_sb[:, :, hsl], in_=wkr[:, :, hsl])
            nc.sync.dma_start(out=cosT, in_=cosT_d)
            nc.sync.dma_start(out=sinN, in_=sinN_d)
            nc.vector.memset(ones_f, 1.0)
            nc.vector.tensor_copy(ones_mat, ones_f)

            # ---------------- Phase A: QKV projections + RoPE (one x pass) ---
            for sc in range(NSC):
                ssl = slice(sc * SC, (sc + 1) * SC)
                if sc == 0:
                    x_sb = x0
                else:
                    x_sb = xpool.tile([128, NHT, SC], BF16, tag="x")
                    nc.sync.dma_start(
                        out=x_sb, in_=xT[:, ssl].rearrange("(t p) s -> p t s", p=128)
                    )
                # V projection: x block stationary -> [s, (h d)] orientation
                for st2 in range(SC // 128):
                    st = sc * (SC // 128) + st2
                    pv = ps.tile([128, GC], F32, tag="a", bufs=2)
                    for ht in range(NHT):
                        nc.tensor.matmul(
                            pv,
                            x_sb[:, ht, st2 * 128 : (st2 + 1) * 128],
                            wv_sb[:, ht, :],
                            start=(ht == 0),
                            stop=(ht == NHT - 1),
                        )
                    nc.scalar.copy(v_all[:, st, :], pv)
                if sc == 2:
                    # Stagger the late-needed woc/tri DMAs behind chunk 2's V
                    # work (WAW dep via dummy writes that depend on v_all) so
                    # the front-loaded wv/x/wq/wk/cos/sin DMAs get the full
                    # bandwidth. tri is needed at phase B start, woc only in
                    # phase C.
                    nc.vector.tensor_copy(woc_sb[0:1, 0, 0:1], v_all[0:1, 11, 0:1])
                    nc.vector.tensor_copy(tri[0:1, 0:1], v_all[0:1, 11, 0:1])
                    nc.sync.dma_start(out=tri, in_=tri_d)
                    nc.sync.dma_start(
                        out=woc_sb, in_=woc.rearrange("(c p) o -> p c o", p=128)
                    )
                # Q/K projections: w block stationary -> [d, s] orientation.
                # Q and K of a head share one [128,1024] PSUM pair tile so
                # phase A uses the same PSUM tag zones as phase B.
                for h in range(G):
                    pqk2 = ps.tile([128, 2 * SC], F32, tag="b", bufs=2)
                    for off, w_sb in ((0, wq_sb), (SC, wk_sb)):
                        pqk = pqk2[:, off : off + SC]
                        for ht in range(NHT):
                            nc.tensor.matmul(
                                pqk,
                                w_sb[:, ht, h * 128 : (h + 1) * 128],
                                x_sb[:, ht, :],
                                start=(ht == 0),
                                stop=(ht == NHT - 1),
                            )
                    for off, dst in ((0, q_all), (SC, k_all)):
                        pqk = pqk2[:, off : off + SC]
                        # RoPE: dst = pqk * cos + rot_half(pqk) * sin
                        tmp = rpool.tile([128, SC], F32, tag="tmp")
                        nc.vector.tensor_tensor(
                            out=tmp[0:64, :], in0=pqk[64:128, :],
                            in1=sinN[0:64, ssl], op=MULT,
                        )
                        nc.vector.tensor_tensor(
                            out=tmp[64:128, :], in0=pqk[0:64, :],
                            in1=sinN[64:128, ssl], op=MULT,
                        )
                        cpart = rpool.tile([128, SC], F32, tag="cpart")
                        nc.vector.tensor_tensor(
                            out=cpart, in0=pqk, in1=cosT[:, ssl], op=MULT
                        )
                        nc.vector.tensor_tensor(
                            out=dst[:, h, ssl], in0=cpart, in1=tmp, op=ADD
                        )

            # ---------------- Phase B: attention (all SBUF-resident) ---------
            # k-blocks are processed in PAIRS packed into one [128, 1024]
            # PSUM tile: one exp instruction per pair (halves the Act-engine
            # per-instruction overhead, Act is the phase-B co-bottleneck).
            # Diagonal pairs pack only the causally-valid column ranges.
            # The pair stream is software-pipelined D=2 deep across chunk and
            # head boundaries so the PE never waits on exp latency.
            pairs = []  # (h, qc, kp, npair)
            for h in range(G):
                for qc in range(NQC):
                    npair = 2 * qc + 2
                    for kp in range(npair):
                        pairs.append((h, qc, kp, npair))
            D = 2
            meta = {}  # pair idx -> (pr, halves)
            acc = {}  # (h, qc) -> (po, pbs)

            def front(j):
                h, qc, kp, npair = pairs[j]
                psc = ps.tile([128, 2 * QC], F32, tag="b", bufs=2)
                pr = prpool.tile([128, 2 * QC], BF16, tag="pr")
                halves = []
                off = 0
                for half in range(2):
                    ki = 2 * kp + half
                    m = ki - 4 * qc  # >= 0 on diagonal blocks
                    c0 = max(m, 0) * 128  # valid col start within chunk
                    w = QC - c0
                    nc.tensor.matmul(
                        psc[:, off : off + w],
                        k_all[:, h, ki * 128 : (ki + 1) * 128],
                        q_all[:, h, qc * QC + c0 : (qc + 1) * QC],
                        start=True,
                        stop=True,
                    )
                    halves.append((ki, m, c0, w, off))
                    off += w
                nc.scalar.activation(
                    pr[:, 0:off], psc[:, 0:off], EXP, scale=INV_SQRT_HD
                )
                for ki, m, c0, w, o in halves:
                    if m >= 0:
                        # triangular mask on the partial sub-block, on the
                        # (otherwise idle) Pool engine
                        nc.gpsimd.tensor_tensor(
                            out=pr[:, o : o + 128],
                            in0=pr[:, o : o + 128],
                            in1=tri, op=MULT,
                        )
                meta[j] = (pr, halves)

            def back(j):
                h, qc, kp, npair = pairs[j]
                hd = slice(h * 128, (h + 1) * 128)
                nk = 4 * qc + 4
                if kp == 0:
                    acc[(h, qc)] = (
                        ps.tile([128, QC], F32, tag="a", bufs=2, name="po"),
                        ps.tile([128, QC], F32, tag="c", bufs=2, name="pbs"),
                    )
                po, pbs = acc[(h, qc)]
                pr, halves = meta.pop(j)
                for ki, m, c0, w, o in halves:
                    nc.tensor.matmul(
                        po[:, c0:QC], v_all[:, ki, hd], pr[:, o : o + w],
                        start=(ki == 0), stop=(ki == nk - 1),
                    )
                    nc.tensor.matmul(
                        pbs[:, c0:QC], ones_mat, pr[:, o : o + w],
                        start=(ki == 0), stop=(ki == nk - 1),
                    )
                if kp == npair - 1:
                    # Act stages po out of PSUM right away (frees the bank);
                    # normalize on DVE behind its own reciprocal — nothing
                    # upstream depends on outh until phase C.
                    pof = bcpool.tile([128, QC], F32, tag="pof")
                    nc.scalar.copy(pof, po)
                    bc = bcpool.tile([128, QC], F32, tag="bc")
                    nc.vector.reciprocal(out=bc, in_=pbs)
                    nc.vector.tensor_tensor(
                        out=outh[:, h, slice(qc * QC, (qc + 1) * QC)],
                        in0=pof, in1=bc, op=MULT,
                    )

            for j in range(len(pairs) + D):
                if j < len(pairs):
                    front(j)
                if j >= D:
                    back(j - D)

            # ---------------- Phase C: output projection ---------------------
            # sch outer so the first 16 py groups only need outh columns that
            # finished early in phase B
            for sch in range(NQC):
                ssl = slice(sch * QC, (sch + 1) * QC)
                for ot in range(NHT):
                    py = ps.tile([128, QC], F32, tag="a", bufs=2)
                    for h in range(G):
                        nc.tensor.matmul(
                            py,
                            woc_sb[:, h, ot * 128 : (ot + 1) * 128],
                            outh[:, h, ssl],
                            start=(h == 0),
                            stop=(h == G - 1),
                        )
                    ysf = ystage.tile([128, QC], BF16)
                    nc.scalar.copy(ysf, py)
                    nc.scalar.dma_start(
                        out=yT[ot * 128 : (ot + 1) * 128, ssl], in_=ysf
                    )
    _split_multi_waits(nc)
    return nc


def _host_tables():
    import ml_dtypes

    inv_freq = 1.0 / (THETA ** (np.arange(0, HD, 2, dtype=np.float32) / HD))
    t = np.arange(S, dtype=np.float32)
    freqs = np.einsum("i,j->ij", t, inv_freq)  # [S, 64]
    cos_h = np.cos(freqs).astype(np.float32)  # [S, 64]
    sin_h = np.sin(freqs).astype(np.float32)
    cosT = np.empty((HD, S), np.float32)
    cosT[0:64] = cos_h.T
    cosT[64:128] = cos_h.T
    sinN = np.empty((HD, S), np.float32)
    sinN[0:64] = -sin_h.T
    sinN[64:128] = sin_h.T
    p = np.arange(128)[:, None]
    s = np.arange(128)[None, :]
    tri = (s >= p).astype(ml_dtypes.bfloat16)
    return (
        cosT.astype(ml_dtypes.bfloat16),
        sinN.astype(ml_dtypes.bfloat16),
        tri,
    )


def kernel(hidden_states, Wq, Wk, Wv, Wo):
    import ml_dtypes

    from concourse import bass_utils

    BF = ml_dtypes.bfloat16
    hidden_states = np.asarray(hidden_states, dtype=np.float32)
    Wq = np.asarray(Wq, dtype=np.float32)
    Wk = np.asarray(Wk, dtype=np.float32)
    Wv = np.asarray(Wv, dtype=np.float32)
    Wo = np.asarray(Wo, dtype=np.float32)

    if "nc" not in _prog_cache:
        _prog_cache["nc"] = _build_nc()
    nc = _prog_cache["nc"]

    cosT, sinN, tri = _host_tables()
    in_maps = []
    for c in range(8):
        b, g = divmod(c, 4)
        rows = slice(g * GC, (g + 1) * GC)
        in_maps.append(
            {
                "xT": np.ascontiguousarray(hidden_states[b].T).astype(BF),
                "wqT": np.ascontiguousarray(Wq[rows, :].T).astype(BF),
                "wkT": np.ascontiguousarray(Wk[rows, :].T).astype(BF),
                "wvT": np.ascontiguousarray(Wv[rows, :].T).astype(BF),
                "woc": np.ascontiguousarray(Wo[:, rows].T).astype(BF),
                "cosT": cosT,
                "sinN": sinN,
                "tri": tri,
            }
        )

    res = bass_utils.run_bass_kernel_spmd(
        nc, in_maps, core_ids=list(range(8)), trace=TRACE
    )
    global LAST_RESULTS
    LAST_RESULTS = res

    out = np.zeros((B, S, H), np.float32)
    for c in range(8):
        b = c // 4
        out[b] += res.results[c]["yT"].T.astype(np.float32)
    return out


# revision 34
# speedup vs baseline: 1.0294x; 1.0263x over previous
"""DharmaAttention TRN2 kernel (fused single-pass, bf16, v3).

Full-input contract: kernel(**inputs) takes the unsharded inputs and returns
the full [2, 2048, 2048] output.

Sharding (8 cores): 2-way data-parallel over batch x 4-way tensor-parallel
over head groups (4 heads of head_dim 128 per core). Wq/Wk/Wv are split
column-wise (output channels) per head group, Wo row-wise; each core produces
a partial output projection for its batch element and the host sums the 4
partials per batch.

v3 changes vs v2:
  - reciprocal_approx_fast instead of reciprocal (3.4us -> 0.7us DVE op that
    was head-of-line blocking the diag mask multiplies -> PE stalls).
  - Diagonal blocks compute only the valid (causal) column range; the
    triangular mask shrinks to a single [128,128] constant applied to one
    sub-block per diagonal block.
  - One PSUM pool with shared tags across phases (no pool-transition
    barriers at phase boundaries).
  - Startup DMAs split/ordered so the first V matmul starts early.
  - bf16 output staged per [128,512] block (halves output DMA, short tail).

Per-core layouts (host-side prep):
  xT   [2048, 2048] bf16  hidden_states[b].T       (contraction dim on partitions)
  wqT  [2048, 512]  bf16  Wq[rows of group].T      (same for wkT, wvT)
  woc  [512, 2048]  bf16  Wo[:, cols of group].T
  cosT [128, 2048]  f32   rope cos table, [d, s]
  sinN [128, 2048]  f32   rows 0:64 = -sin, rows 64:128 = +sin, [d, s]
  tri  [128, 128]   bf16  tri[p, s] = 1 if s >= p (in-block causal mask)
Output:
  yT   [2048, 2048] bf16  partial (Wo row-shard) output, transposed [o, s]

Softmax skips the max subtraction: scores are O(+-6), exp is safe in fp32,
and softmax is shift-invariant so the result matches the reference.
"""

import math
import sys

sys.path.insert(0, "/opt/trn_rl_repo")

import numpy as np

B = 2
S = 2048
H = 2048
NH = 16
HD = 128
THETA = 10000.0
G = 4  # heads per core (tensor-parallel group size NH / 4)
GC = G * HD  # channels per core = 512
NHT = H // 128  # 16 contraction tiles
SC = 512  # projection seq chunk
NSC = S // SC  # 4
QC = 512  # attention q chunk
NQC = S // QC  # 4
NKB = S // 128  # 16 k blocks
INV_SQRT_HD = 1.0 / math.sqrt(HD)

_prog_cache = {}

# test-harness hooks (the grading path leaves these at defaults)
TRACE = False
LAST_RESULTS = None


def _split_multi_waits(nc):
    """The walrus build here accepts at most ONE sync wait per instruction
    ('Too many sync wait commands'). Hoist extra on_wait entries into no-op
    instructions inserted just before, on the same engine."""
    import concourse.mybir as mybir

    for f in nc.m.functions:
        for b in f.blocks:
            out = []
            changed = False
            for inst in b.instructions:
                si = getattr(inst, "sync_info", None)
                waits = list(si.on_wait) if si is not None and si.on_wait else []
                if len(waits) > 1:
                    for k, w in enumerate(waits[:-1]):
                        nop = mybir.InstNoOp(
                            name=f"{inst.name}-w{k}",
                            sync_info=mybir.SyncInfo(on_wait=[w], on_update=[]),
                        )
                        nop.engine = inst.engine
                        out.append(nop)
                    inst.sync_info = mybir.SyncInfo(
                        on_wait=[waits[-1]], on_update=list(si.on_update or [])
                    )
                    changed = True
                out.append(inst)
            if changed:
                b.instructions = out
    return nc


def _build_nc():
    import concourse.bass as bass
    import concourse.mybir as mybir
    import concourse.tile as tile

    F32 = mybir.dt.float32
    BF16 = mybir.dt.bfloat16
    MULT = mybir.AluOpType.mult
    ADD = mybir.AluOpType.add
    DIV = mybir.AluOpType.divide
    EXP = mybir.ActivationFunctionType.Exp

    nc = bass.Bass("TRN2", target_bir_lowering=False, debug=False)

    # All inputs pre-packed on the host into the exact SBUF layouts so every
    # DMA is a large contiguous copy (strided gathers splinter into ~0.5-1KB
    # descriptors and the 16 DMA queues become descriptor-bound at startup).
    xP = nc.dram_tensor("xP", [NSC, 128, NHT, SC], BF16, kind="ExternalInput").ap()
    wqP = nc.dram_tensor("wqP", [128, G, NHT, 128], BF16, kind="ExternalInput").ap()
    wkP = nc.dram_tensor("wkP", [128, G, NHT, 128], BF16, kind="ExternalInput").ap()
    wvP = nc.dram_tensor("wvP", [128, NHT, GC], BF16, kind="ExternalInput").ap()
    wocP = nc.dram_tensor("wocP", [128, G, H], BF16, kind="ExternalInput").ap()
    cosT_d = nc.dram_tensor("cosT", [HD, S], BF16, kind="ExternalInput").ap()
    sinN_d = nc.dram_tensor("sinN", [HD, S], BF16, kind="ExternalInput").ap()
    tri_d = nc.dram_tensor("tri", [128, 128], BF16, kind="ExternalInput").ap()
    yP = nc.dram_tensor("yP", [NQC, NHT, 128, QC], BF16, kind="ExternalOutput").ap()

    with tile.TileContext(nc) as tc:
        with (
            tc.tile_pool(name="consts", bufs=1) as consts,
            tc.tile_pool(name="qkv", bufs=1) as qkv,
            tc.tile_pool(name="wpool", bufs=1) as wpool,
            tc.tile_pool(name="xpool", bufs=2) as xpool,
            tc.tile_pool(name="rpool", bufs=3) as rpool,
            tc.tile_pool(name="prpool", bufs=4) as prpool,
            tc.tile_pool(name="bcpool", bufs=2) as bcpool,
            tc.tile_pool(name="ystage", bufs=4) as ystage,
            tc.tile_pool(name="ps", bufs=1, space="PSUM") as ps,
        ):
            # persistent SBUF state for the whole kernel
            cosT = consts.tile([HD, S], BF16)
            sinN = consts.tile([HD, S], BF16)
            tri = consts.tile([128, 128], BF16)
            ones_f = consts.tile([128, 128], F32)
            ones_mat = consts.tile([128, 128], BF16)
            woc_sb = consts.tile([128, G, H], BF16, tag="woc")

            q_all = qkv.tile([128, G, S], BF16, tag="q")  # [d, h, s]
            k_all = qkv.tile([128, G, S], BF16, tag="k")  # [d, h, s]
            v_all = qkv.tile([128, NKB, GC], BF16, tag="v")  # [s_in_blk, blk, (h d)]
            outh = qkv.tile([128, G, S], BF16, tag="o")  # [d, h, s]

            wv_sb = wpool.tile([128, NHT, GC], BF16, tag="wv")
            wq_sb = wpool.tile([128, G, NHT, 128], BF16, tag="wq")
            wk_sb = wpool.tile([128, G, NHT, 128], BF16, tag="wk")
            x0 = xpool.tile([128, NHT, SC], BF16, tag="x")

            # startup DMAs, interleaved so the first V matmul group (which
            # needs all of wv + x0) completes as early as possible, with
            # later-needed tensors queued behind.
            for c in range(4):
                tsl = slice(4 * c, 4 * c + 4)
                nc.sync.dma_start(out=wv_sb[:, tsl, :], in_=wvP[:, tsl, :])
                nc.sync.dma_start(out=x0[:, tsl, :], in_=xP[0][:, tsl, :])
            # wq/wk split per head, interleaved in the order phase A consumes
            # them (Q head 0, K head 0, Q head 1, ...)
            for h in range(G):
                nc.sync.dma_start(out=wq_sb[:, h], in_=wqP[:, h])
                nc.sync.dma_start(out=wk_sb[:, h], in_=wkP[:, h])
            nc.sync.dma_start(out=cosT, in_=cosT_d)
            nc.sync.dma_start(out=sinN, in_=sinN_d)
            nc.vector.memset(ones_f, 1.0)
            nc.vector.tensor_copy(ones_mat, ones_f)

            # ---------------- Phase A: QKV projections + RoPE (one x pass) ---
            for sc in range(NSC):
                ssl = slice(sc * SC, (sc + 1) * SC)
                if sc == 0:
                    x_sb = x0
                else:
                    x_sb = xpool.tile([128, NHT, SC], BF16, tag="x")
                    nc.sync.dma_start(out=x_sb, in_=xP[sc])
                # V projection: x block stationary -> [s, (h d)] orientation
                for st2 in range(SC // 128):
                    st = sc * (SC // 128) + st2
                    pv = ps.tile([128, GC], F32, tag="a", bufs=2)
                    for ht in range(NHT):
                        nc.tensor.matmul(
                            pv,
                            x_sb[:, ht, st2 * 128 : (st2 + 1) * 128],
                            wv_sb[:, ht, :],
                            start=(ht == 0),
                            stop=(ht == NHT - 1),
                        )
                    nc.scalar.copy(v_all[:, st, :], pv)
                if sc == 2:
                    # Stagger the late-needed woc/tri DMAs behind chunk 2's V
                    # work (WAW dep via dummy writes that depend on v_all) so
                    # the front-loaded wv/x/wq/wk/cos/sin DMAs get the full
                    # bandwidth. tri is needed at phase B start, woc only in
                    # phase C.
                    nc.vector.tensor_copy(woc_sb[0:1, 0, 0:1], v_all[0:1, 11, 0:1])
                    nc.vector.tensor_copy(tri[0:1, 0:1], v_all[0:1, 11, 0:1])
                    nc.sync.dma_start(out=tri, in_=tri_d)
                    nc.sync.dma_start(out=woc_sb, in_=wocP)
                # Q/K projections: w block stationary -> [d, s] orientation.
                # Q and K of a head share one [128,1024] PSUM pair tile so
                # phase A uses the same PSUM tag zones as phase B.
                for h in range(G):
                    pqk2 = ps.tile([128, 2 * SC], F32, tag="b", bufs=2)
                    for off, w_sb in ((0, wq_sb), (SC, wk_sb)):
                        pqk = pqk2[:, off : off + SC]
                        for ht in range(NHT):
                            nc.tensor.matmul(
                                pqk,
                                w_sb[:, h, ht, :],
                                x_sb[:, ht, :],
                                start=(ht == 0),
                                stop=(ht == NHT - 1),
                            )
                    for off, dst in ((0, q_all), (SC, k_all)):
                        pqk = pqk2[:, off : off + SC]
                        # RoPE: dst = pqk * cos + rot_half(pqk) * sin
                        tmp = rpool.tile([128, SC], F32, tag="tmp")
                        nc.vector.tensor_tensor(
                            out=tmp[0:64, :], in0=pqk[64:128, :],
                            in1=sinN[0:64, ssl], op=MULT,
                        )
                        nc.vector.tensor_tensor(
                            out=tmp[64:128, :], in0=pqk[0:64, :],
                            in1=sinN[64:128, ssl], op=MULT,
                        )
                        cpart = rpool.tile([128, SC], F32, tag="cpart")
                        nc.vector.tensor_tensor(
                            out=cpart, in0=pqk, in1=cosT[:, ssl], op=MULT
                        )
                        nc.vector.tensor_tensor(
                            out=dst[:, h, ssl], in0=cpart, in1=tmp, op=ADD
                        )

            # ---------------- Phase B: attention (all SBUF-resident) ---------
            # k-blocks are processed in PAIRS packed into one [128, 1024]
            # PSUM tile: one exp instruction per pair (halves the Act-engine
            # per-instruction overhead, Act is the phase-B co-bottleneck).
            # Diagonal pairs pack only the causally-valid column ranges.
            # The pair stream is software-pipelined D=2 deep across chunk and
            # head boundaries so the PE never waits on exp latency.
            pairs = []  # (h, qc, kp, npair)
            for h in range(G):
                for qc in range(NQC):
                    npair = 2 * qc + 2
                    for kp in range(npair):
                        pairs.append((h, qc, kp, npair))
            D = 2
            meta = {}  # pair idx -> (pr, halves)
            acc = {}  # (h, qc) -> (po, pbs)

            def front(j):
                h, qc, kp, npair = pairs[j]
                psc = ps.tile([128, 2 * QC], F32, tag="b", bufs=2)
                pr = prpool.tile([128, 2 * QC], BF16, tag="pr")
                halves = []
                off = 0
                for half in range(2):
                    ki = 2 * kp + half
                    m = ki - 4 * qc  # >= 0 on diagonal blocks
                    c0 = max(m, 0) * 128  # valid col start within chunk
                    w = QC - c0
                    nc.tensor.matmul(
                        psc[:, off : off + w],
                        k_all[:, h, ki * 128 : (ki + 1) * 128],
                        q_all[:, h, qc * QC + c0 : (qc + 1) * QC],
                        start=True,
                        stop=True,
                    )
                    halves.append((ki, m, c0, w, off))
                    off += w
                nc.scalar.activation(
                    pr[:, 0:off], psc[:, 0:off], EXP, scale=INV_SQRT_HD
                )
                for ki, m, c0, w, o in halves:
                    if m >= 0:
                        # triangular mask on the partial sub-block, on the
                        # (otherwise idle) Pool engine
                        nc.gpsimd.tensor_tensor(
                            out=pr[:, o : o + 128],
                            in0=pr[:, o : o + 128],
                            in1=tri, op=MULT,
                        )
                meta[j] = (pr, halves)

            def back(j):
                h, qc, kp, npair = pairs[j]
                hd = slice(h * 128, (h + 1) * 128)
                nk = 4 * qc + 4
                if kp == 0:
                    acc[(h, qc)] = (
                        ps.tile([128, QC], F32, tag="a", bufs=2, name="po"),
                        ps.tile([128, QC], F32, tag="c", bufs=2, name="pbs"),
                    )
                po, pbs = acc[(h, qc)]
                pr, halves = meta.pop(j)
                for ki, m, c0, w, o in halves:
                    nc.tensor.matmul(
                        po[:, c0:QC], v_all[:, ki, hd], pr[:, o : o + w],
                        start=(ki == 0), stop=(ki == nk - 1),
                    )
                    nc.tensor.matmul(
                        pbs[:, c0:QC], ones_mat, pr[:, o : o + w],
                        start=(ki == 0), stop=(ki == nk - 1),
                    )
                if kp == npair - 1:
                    # Act stages po out of PSUM right away (frees the bank);
                    # normalize on DVE behind its own reciprocal — nothing
                    # upstream depends on outh until phase C.
                    pof = bcpool.tile([128, QC], F32, tag="pof")
                    nc.scalar.copy(pof, po)
                    bc = bcpool.tile([128, QC], F32, tag="bc")
                    nc.vector.reciprocal(out=bc, in_=pbs)
                    nc.vector.tensor_tensor(
                        out=outh[:, h, slice(qc * QC, (qc + 1) * QC)],
                        in0=pof, in1=bc, op=MULT,
                    )

            for j in range(len(pairs) + D):
                if j < len(pairs):
                    front(j)
                if j >= D:
                    back(j - D)

            # ---------------- Phase C: output projection ---------------------
            # sch outer so the first 16 py groups only need outh columns that
            # finished early in phase B
            for sch in range(NQC):
                ssl = slice(sch * QC, (sch + 1) * QC)
                for ot in range(NHT):
                    py = ps.tile([128, QC], F32, tag="a", bufs=2)
                    for h in range(G):
                        nc.tensor.matmul(
                            py,
                            woc_sb[:, h, ot * 128 : (ot + 1) * 128],
                            outh[:, h, ssl],
                            start=(h == 0),
                            stop=(h == G - 1),
                        )
                    ysf = ystage.tile([128, QC], BF16)
                    nc.scalar.copy(ysf, py)
                    nc.scalar.dma_start(out=yP[sch, ot], in_=ysf)
    _split_multi_waits(nc)
    return nc


def _host_tables():
    import ml_dtypes

    inv_freq = 1.0 / (THETA ** (np.arange(0, HD, 2, dtype=np.float32) / HD))
    t = np.arange(S, dtype=np.float32)
    freqs = np.einsum("i,j->ij", t, inv_freq)  # [S, 64]
    cos_h = np.cos(freqs).astype(np.float32)  # [S, 64]
    sin_h = np.sin(freqs).astype(np.float32)
    cosT = np.empty((HD, S), np.float32)
    cosT[0:64] = cos_h.T
    cosT[64:128] = cos_h.T
    sinN = np.empty((HD, S), np.float32)
    sinN[0:64] = -sin_h.T
    sinN[64:128] = sin_h.T
    p = np.arange(128)[:, None]
    s = np.arange(128)[None, :]
    tri = (s >= p).astype(ml_dtypes.bfloat16)
    return (
        cosT.astype(ml_dtypes.bfloat16),
        sinN.astype(ml_dtypes.bfloat16),
        tri,
    )


def _pack_core(x, Wq, Wk, Wv, Wo, g):
    """Pack one core's inputs into the exact SBUF layouts (contiguous DMAs)."""
    import ml_dtypes

    BF = ml_dtypes.bfloat16
    rows = slice(g * GC, (g + 1) * GC)
    xT = x.T.astype(BF)  # [H, S]
    xP = np.ascontiguousarray(xT.reshape(NHT, 128, NSC, SC).transpose(2, 1, 0, 3))

    def wqk(W):
        wT = W[rows, :].T.astype(BF)  # [H, GC]
        return np.ascontiguousarray(
            wT.reshape(NHT, 128, G, 128).transpose(1, 2, 0, 3)
        )

    wvT = Wv[rows, :].T.astype(BF)
    wvP = np.ascontiguousarray(wvT.reshape(NHT, 128, GC).transpose(1, 0, 2))
    woT = Wo[:, rows].T.astype(BF)  # [GC, H]
    wocP = np.ascontiguousarray(woT.reshape(G, 128, H).transpose(1, 0, 2))
    return {"xP": xP, "wqP": wqk(Wq), "wkP": wqk(Wk), "wvP": wvP, "wocP": wocP}


def kernel(hidden_states, Wq, Wk, Wv, Wo):
    import ml_dtypes

    from concourse import bass_utils

    BF = ml_dtypes.bfloat16
    hidden_states = np.asarray(hidden_states, dtype=np.float32)
    Wq = np.asarray(Wq, dtype=np.float32)
    Wk = np.asarray(Wk, dtype=np.float32)
    Wv = np.asarray(Wv, dtype=np.float32)
    Wo = np.asarray(Wo, dtype=np.float32)

    if "nc" not in _prog_cache:
        _prog_cache["nc"] = _build_nc()
    nc = _prog_cache["nc"]

    cosT, sinN, tri = _host_tables()
    in_maps = []
    for c in range(8):
        b, g = divmod(c, 4)
        in_maps.append(
            {
                **_pack_core(hidden_states[b], Wq, Wk, Wv, Wo, g),
                "cosT": cosT,
                "sinN": sinN,
                "tri": tri,
            }
        )

    res = bass_utils.run_bass_kernel_spmd(
        nc, in_maps, core_ids=list(range(8)), trace=TRACE
    )
    global LAST_RESULTS
    LAST_RESULTS = res

    out = np.zeros((B, S, H), np.float32)
    for c in range(8):
        b = c // 4
        # yP [NQC sch, NHT ot, 128 p, QC s] -> [S, H]
        yP = res.results[c]["yP"].astype(np.float32)
        out[b] += yP.transpose(0, 3, 1, 2).reshape(S, H)
    return out
